# revision 1
# baseline (speedup 1.0000x reference)
"""Trainium2 Bass kernel for nn_BasicLSTM: fc0 -> 10x LSTM(768) -> fc1.

Strategy: data-parallel over the 512 windows across 8 cores (64 windows each).
All matmul operands in bf16 (f32 PSUM accumulation, f32 elementwise/state);
measured end-to-end L2 relative error vs the f32 reference: ~1.8e-3.

Per-core structure (per layer l):
  phase-gx : batched GEMM gx = X_{l-1}^T.T @ w_ihT + bias over all T steps.
             X^T stationary (M=128 tiles), w_ihT streamed as moving chunks.
             Output rows are (t, b) pairs: m-tile m holds steps 2m (partitions
             0-63) and 2m+1 (64-127); odd steps are engine-copied to a staging
             tile and DMA partition-shifted to base 0 (gxO).
  phase-rec: per step t: g = gx[t] + h^T.T @ w_hhT (h^T stationary M=64,
             w_hhT resident in SBUF as moving chunks), gates on ACT, c/h on
             DVE, h transposed back into X^T via PE transpose.
  X^T is a single (128, KT, T*64) buffer reused across layers (phase-gx fully
  consumes layer l-1's output before phase-rec overwrites it).
fc1 contracts X^T_last over all (t, h) with streamed fc1_w chunks.
"""
import numpy as np
import ml_dtypes

H = 768
G = 4 * H          # 3072
W = 10             # time steps (window size)
L = 10             # layers
B_FULL = 512
NCORES = 8
BL = B_FULL // NCORES  # 64 windows per core

_CACHE = {}


def build_program(h=H, w=W, nl=L, bl=BL, variant="full"):
    import concourse.mybir as mybir
    import concourse.tile as tile
    from concourse import bacc
    from concourse.masks import make_identity

    F32 = mybir.dt.float32
    BF16 = mybir.dt.bfloat16
    AF = mybir.ActivationFunctionType
    OP = mybir.AluOpType

    g4 = 4 * h
    kt = h // 128           # k-tiles over h
    nch = g4 // 512         # 512-wide chunks over the gate dim
    mt = (w * bl) // 128    # m-tiles over the (t, b) axis
    fh = w * h              # fc1 contraction size
    fn1 = h // 2            # fc1 output chunk (two psum chunks)
    assert h % 128 == 0 and g4 % 512 == 0 and (w * bl) % 128 == 0 and bl == 64

    nc = bacc.Bacc("TRN2", target_bir_lowering=False, debug=False)

    xposT_d = nc.dram_tensor("xposT", [h, w * bl], BF16, kind="ExternalInput")
    fc0wT_d = nc.dram_tensor("fc0wT", [h, h], BF16, kind="ExternalInput")
    fc0b_d = nc.dram_tensor("fc0b", [1, h], BF16, kind="ExternalInput")
    wihT_d = nc.dram_tensor("wihT", [nl, h, g4], BF16, kind="ExternalInput")
    whhT_d = nc.dram_tensor("whhT", [nl, h, g4], BF16, kind="ExternalInput")
    biasT_d = nc.dram_tensor("biasT", [nl, 1, g4], BF16, kind="ExternalInput")
    fc1wT_d = nc.dram_tensor("fc1wT", [fh, h], BF16, kind="ExternalInput")
    fc1bT_d = nc.dram_tensor("fc1bT", [1, h], BF16, kind="ExternalInput")
    out_d = nc.dram_tensor("out", [bl, h], F32, kind="ExternalOutput")

    with tile.TileContext(nc) as tc, \
         tc.tile_pool(name="persist", bufs=1) as pp, \
         tc.tile_pool(name="gxp", bufs=1) as gxp, \
         tc.tile_pool(name="wstream", bufs=6) as wsp, \
         tc.tile_pool(name="stag", bufs=2) as stp, \
         tc.tile_pool(name="gpool", bufs=1) as gp, \
         tc.tile_pool(name="gatep", bufs=1) as gatep, \
         tc.tile_pool(name="tmp", bufs=2) as tp, \
         tc.tile_pool(name="cpool", bufs=2) as cp, \
         tc.tile_pool(name="hpool", bufs=2) as hp, \
         tc.tile_pool(name="psA", bufs=6, space="PSUM") as psA, \
         tc.tile_pool(name="psT", bufs=2, space="PSUM") as psT:

        # ---- persistent tiles ----
        XT = pp.tile([128, kt, w * bl], BF16)      # h^T / layer-input storage
        WHH = pp.tile([128, kt, g4], BF16)         # resident w_hhT
        onesb = pp.tile([1, 512], BF16)
        nc.vector.memset(onesb[:], 1.0)
        idb = pp.tile([64, 64], BF16)
        make_identity(nc, idb[:])
        fc0b_sb = pp.tile([1, h], BF16)
        nc.sync.dma_start(fc0b_sb[:], fc0b_d[:])
        fc1b_sb = pp.tile([1, h], BF16)
        nc.sync.dma_start(fc1b_sb[:], fc1bT_d[:])

        # ---- fc0: XT <- fc0_wT.T @ xposT + fc0_b ----
        fc0w = gxp.tile([128, kt, h], BF16, tag="gxE")   # reuses gxE slot space
        nc.sync.dma_start(fc0w[:], fc0wT_d.rearrange("(k p) ho -> p k ho", p=128))
        xpt = gxp.tile([128, kt, w * bl], BF16, tag="gxO")  # reuses gxO slot space
        nc.sync.dma_start(xpt[:], xposT_d.rearrange("(k p) c -> p k c", p=128))
        fc0_chunks = [(c, min(512, w * bl - c)) for c in range(0, w * bl, 512)]
        for m in range(kt):
            for c0, cw in fc0_chunks:
                ps = psA.tile([128, 512], F32, tag="psA", name=f"fc0ps_{m}_{c0}")
                for k in range(kt):
                    nc.tensor.matmul(
                        ps[:, :cw],
                        fc0w[:, k, m * 128:(m + 1) * 128],
                        xpt[:, k, c0:c0 + cw],
                        start=(k == 0), stop=False,
                    )
                nc.tensor.matmul(
                    ps[:, :cw], fc0b_sb[:, m * 128:(m + 1) * 128],
                    onesb[:, 0:cw], start=False, stop=True)
                nc.vector.tensor_copy(XT[:, m, c0:c0 + cw], ps[:, :cw])

        # ---- layers ----
        for l in range(nl):
            nc.sync.dma_start(WHH[:], whhT_d[l].rearrange("(k p) g -> p k g", p=128))
            bias_sb = pp.tile([1, g4], BF16, tag="bias")
            nc.sync.dma_start(bias_sb[:], biasT_d[l])

            # --- phase-gx ---
            gxE = gxp.tile([64, mt, g4], BF16, tag="gxE")  # even steps (2m)
            gxO = gxp.tile([64, mt, g4], BF16, tag="gxO")  # odd steps (2m+1)
            for j in range(nch):
                js = slice(j * 512, (j + 1) * 512)
                pss = [psA.tile([128, 512], F32, tag="psA", name=f"gxps_{l}_{j}_{m}")
                       for m in range(mt)]
                for k in range(kt):
                    wk = wsp.tile([128, 512], BF16, tag="wch")
                    nc.sync.dma_start(
                        wk[:],
                        wihT_d[l].rearrange("(k p) g -> p k g", p=128)[:, k, js])
                    for m in range(mt):
                        nc.tensor.matmul(
                            pss[m][:], XT[:, k, m * 128:(m + 1) * 128], wk[:],
                            start=(k == 0), stop=False)
                for m in range(mt):
                    nc.tensor.matmul(
                        pss[m][:], onesb[:, 0:128], bias_sb[:, js], start=False, stop=True)
                for m in range(mt):
                    nc.vector.tensor_copy(gxE[:, m, js], pss[m][0:64, :])
                    stag = stp.tile([128, 512], BF16)
                    nc.vector.tensor_copy(stag[64:128, :], pss[m][64:128, :])
                    nc.sync.dma_start(gxO[:, m, js], stag[64:128, :])

            # --- phase-rec ---
            c_cur = None
            for t in range(w):
                m, half = t // 2, t % 2
                gx_t = (gxE if half == 0 else gxO)[:, m, :]  # (64, g4) bf16
                if t == 0:
                    gsrc = gx_t
                else:
                    g = gp.tile([64, g4], F32)
                    for j in range(nch):
                        js = slice(j * 512, (j + 1) * 512)
                        ps = psA.tile([128, 512], F32, tag="psA", name=f"recps_{l}_{t}_{j}")
                        for k in range(kt):
                            nc.tensor.matmul(
                                ps[0:64, :],
                                XT[:, k, (t - 1) * 64:t * 64],
                                WHH[:, k, js],
                                start=(k == 0), stop=(k == kt - 1))
                        nc.vector.tensor_tensor(
                            g[:, js], ps[0:64, :], gx_t[:, js], OP.add)
                    gsrc = g
                gates = gatep.tile([64, g4], F32)
                TANH = AF.Sigmoid if variant == "allsig" else AF.Tanh
                nc.scalar.activation(gates[:, 0:h], gsrc[:, 0:h], AF.Sigmoid)
                nc.scalar.activation(gates[:, h:2 * h], gsrc[:, h:2 * h], AF.Sigmoid)
                nc.scalar.activation(gates[:, 2 * h:3 * h], gsrc[:, 2 * h:3 * h], TANH)
                nc.scalar.activation(gates[:, 3 * h:4 * h], gsrc[:, 3 * h:4 * h], AF.Sigmoid)
                c_new = cp.tile([64, h], F32)
                if t == 0:
                    nc.vector.tensor_tensor(
                        c_new[:], gates[:, 0:h], gates[:, 2 * h:3 * h], OP.mult)
                else:
                    t1 = tp.tile([64, h], F32, tag="t1")
                    nc.vector.tensor_tensor(
                        t1[:], gates[:, 0:h], gates[:, 2 * h:3 * h], OP.mult)
                    t2 = tp.tile([64, h], F32, tag="t2")
                    nc.vector.tensor_tensor(t2[:], gates[:, h:2 * h], c_cur[:], OP.mult)
                    nc.vector.tensor_tensor(c_new[:], t1[:], t2[:], OP.add)
                c_cur = c_new
                tc_t = tp.tile([64, h], F32, tag="tc")
                nc.scalar.activation(tc_t[:], c_new[:], AF.Sigmoid if variant == "allsig" else AF.Tanh)
                hh = hp.tile([64, h], BF16)
                nc.vector.tensor_tensor(hh[:], gates[:, 3 * h:4 * h], tc_t[:], OP.mult)
                if variant in ("notrans", "noew"):
                    pass  # leave XT stale; timing-structure experiment only
                else:
                    trp = psT.tile([128, kt * 64], BF16)
                    for s in range(kt):
                        nc.tensor.transpose(
                            trp[:, s * 64:(s + 1) * 64], hh[:, s * 128:(s + 1) * 128], idb[:])
                    nc.vector.tensor_copy(
                        XT[:, :, t * 64:(t + 1) * 64],
                        trp[:].rearrange("p (s x) -> p s x", s=kt))

        # ---- fc1 ----
        out_sb = pp.tile([64, h], F32)
        for nchunk in range(2):
            ns = slice(nchunk * fn1, (nchunk + 1) * fn1)
            ps = psA.tile([128, 512], F32, tag="psA", name=f"fc1ps_{nchunk}")
            for ktile in range(w * kt):
                t, s = ktile // kt, ktile % kt
                wk = wsp.tile([128, fn1], BF16, tag="fc1w")
                nc.sync.dma_start(wk[:], fc1wT_d[ktile * 128:(ktile + 1) * 128, ns])
                nc.tensor.matmul(
                    ps[0:64, :fn1], XT[:, s, t * 64:(t + 1) * 64], wk[:],
                    start=(ktile == 0), stop=False)
            nc.tensor.matmul(
                ps[0:64, :fn1], onesb[:, 0:64], fc1b_sb[:, ns], start=False, stop=True)
            nc.vector.tensor_copy(out_sb[:, ns], ps[0:64, :fn1])
        nc.sync.dma_start(out_d[:], out_sb[:])

    nc.compile()
    return nc


def prep_inputs_one(inputs, h=H, w=W, nl=L, bl=BL, core=None, x_batch=None):
    """Host-side prep for one shard. inputs keyed as in setup_inputs()."""
    f32 = np.float32
    bf16 = ml_dtypes.bfloat16
    g4 = 4 * h
    kt = h // 128
    if x_batch is None:
        x = np.ascontiguousarray(np.asarray(inputs["x_position"], f32)).reshape(-1, w, h)
        x_batch = x[core * bl:(core + 1) * bl]
    xposT = np.ascontiguousarray(x_batch.transpose(2, 1, 0).reshape(h, w * bl))
    return {
        "xposT": xposT.astype(bf16),
        "fc0wT": np.ascontiguousarray(np.asarray(inputs["fc0_w"], f32).T).astype(bf16),
        "fc0b": np.ascontiguousarray(
            np.asarray(inputs["fc0_b"], f32).reshape(1, h)).astype(bf16),
        "wihT": np.ascontiguousarray(
            np.asarray(inputs["w_ih"], f32).transpose(0, 2, 1)).astype(bf16),
        "whhT": np.ascontiguousarray(
            np.asarray(inputs["w_hh"], f32).transpose(0, 2, 1)).astype(bf16),
        "biasT": np.ascontiguousarray(
            (np.asarray(inputs["b_ih"], f32) + np.asarray(inputs["b_hh"], f32))
            .reshape(nl, 1, g4)).astype(bf16),
        "fc1wT": np.ascontiguousarray(np.asarray(inputs["fc1_w"], f32).T).astype(bf16),
        "fc1bT": np.ascontiguousarray(
            np.asarray(inputs["fc1_b"], f32).reshape(1, h)).astype(bf16),
    }


def prep_inputs(inputs):
    shared = prep_inputs_one(inputs, core=0)
    x = np.ascontiguousarray(np.asarray(inputs["x_position"], np.float32)).reshape(-1, W, H)
    in_maps = [dict(shared)]
    for c in range(1, NCORES):
        m = dict(shared)
        xc = x[c * BL:(c + 1) * BL]
        m["xposT"] = np.ascontiguousarray(
            xc.transpose(2, 1, 0).reshape(H, W * BL)).astype(ml_dtypes.bfloat16)
        in_maps.append(m)
    return in_maps


def run_on_cores(in_maps, trace=False, **kwargs):
    from concourse.bass_utils import run_bass_kernel_spmd
    if "nc" not in _CACHE:
        _CACHE["nc"] = build_program()
    nc = _CACHE["nc"]
    return run_bass_kernel_spmd(
        nc, in_maps, core_ids=list(range(NCORES)), trace=trace, **kwargs)


def kernel(**inputs) -> np.ndarray:
    in_maps = prep_inputs(inputs)
    res = run_on_cores(in_maps)
    outs = [res.results[c]["out"] for c in range(NCORES)]
    full = np.concatenate(outs, axis=0)            # (512, 768)
    return np.ascontiguousarray(full.reshape(-1, 3).astype(np.float32))



# revision 2
# speedup vs baseline: 1.1297x; 1.1297x over previous
"""Trainium2 Bass kernel for nn_BasicLSTM: fc0 -> 10x LSTM(768) -> fc1.

Strategy: data-parallel over the 512 windows across 8 cores (64 windows each).
All matmul operands in bf16 (f32 PSUM accumulation, f32 elementwise/state).

All 9 logical inputs are packed host-side into ONE bf16 DRAM buffer per core
(per-execute dispatch overhead in this runtime scales with the number of
large argument buffers, ~0.85ms each; one packed buffer pays it once).

Per-core structure (per layer l):
  phase-gx : batched GEMM gx = X_{l-1}^T.T @ w_ihT + bias over all T steps.
             X^T stationary (M=128 tiles), w_ihT streamed as moving chunks.
             Output rows are (t, b) pairs: m-tile m holds steps 2m (partitions
             0-63) and 2m+1 (64-127); odd steps are engine-copied to a staging
             tile and DMA partition-shifted to base 0 (gxO).
  phase-rec: per step t: g = gx[t] + h^T.T @ w_hhT (h^T stationary M=64,
             w_hhT resident in SBUF as moving chunks), gates on ACT, c/h on
             DVE, h transposed back into X^T via PE transpose.
  X^T is a single (128, KT, T*64) buffer reused across layers (phase-gx fully
  consumes layer l-1's output before phase-rec overwrites it).
fc1 contracts X^T_last over all (t, h) with streamed fc1_w chunks.
"""
import numpy as np
import ml_dtypes

H = 768
G = 4 * H          # 3072
W = 10             # time steps (window size)
L = 10             # layers
B_FULL = 512
NCORES = 8
BL = B_FULL // NCORES  # 64 windows per core

# ---- packed-input layout (bf16 element offsets) ----
_SIZES = [
    ("xposT", H * W * BL),        # (H, W*BL)
    ("fc0wT", H * H),             # (H, H)
    ("fc0b", H),                  # (1, H)
    ("wihT", L * H * G),          # (L, H, G)
    ("whhT", L * H * G),          # (L, H, G)
    ("biasT", L * G),             # (L, 1, G)
    ("fc1wT", W * H * H),         # (W*H, H)
    ("fc1bT", H),                 # (1, H)
]
OFF = {}
_c = 0
for _n, _s in _SIZES:
    OFF[_n] = _c
    _c += _s
TOTAL_ELEMS = _c

_CACHE = {}


def build_program(h=H, w=W, nl=L, bl=BL, variant="full"):
    import concourse.mybir as mybir
    import concourse.tile as tile
    from concourse import bacc
    from concourse.masks import make_identity

    F32 = mybir.dt.float32
    BF16 = mybir.dt.bfloat16
    AF = mybir.ActivationFunctionType
    OP = mybir.AluOpType

    g4 = 4 * h
    kt = h // 128           # k-tiles over h
    nch = g4 // 512         # 512-wide chunks over the gate dim
    mt = (w * bl) // 128    # m-tiles over the (t, b) axis
    fh = w * h              # fc1 contraction size
    fn1 = h // 2            # fc1 output chunk (two psum chunks)
    assert h % 128 == 0 and g4 % 512 == 0 and (w * bl) % 128 == 0 and bl == 64

    nc = bacc.Bacc("TRN2", target_bir_lowering=False, debug=False)

    pk = nc.dram_tensor("packed", [TOTAL_ELEMS], BF16, kind="ExternalInput")
    out_d = nc.dram_tensor("out", [bl, h], F32, kind="ExternalOutput")

    def pv(name, n, pattern=None, **axes):
        ap = pk[OFF[name]:OFF[name] + n]
        return ap.rearrange(pattern, **axes) if pattern else ap

    # views into the packed buffer
    xposT_v = pv("xposT", h * w * bl, "(k p c) -> p k c", p=128, c=w * bl)
    fc0wT_v = pv("fc0wT", h * h, "(k p ho) -> p k ho", p=128, ho=h)
    fc0b_v = pv("fc0b", h, "(a ho) -> a ho", a=1)
    fc1bT_v = pv("fc1bT", h, "(a ho) -> a ho", a=1)

    def wihT_v(l, k, js):
        base = OFF["wihT"] + l * h * g4
        ap = pk[base:base + h * g4].rearrange("(k p g) -> p k g", p=128, g=g4)
        return ap[:, k, js]

    def whhT_v(l):
        base = OFF["whhT"] + l * h * g4
        return pk[base:base + h * g4].rearrange("(k p g) -> p k g", p=128, g=g4)

    def biasT_v(l):
        base = OFF["biasT"] + l * g4
        return pk[base:base + g4].rearrange("(a g) -> a g", a=1)

    def fc1wT_v(ktile, ns):
        base = OFF["fc1wT"] + ktile * 128 * h
        ap = pk[base:base + 128 * h].rearrange("(r c) -> r c", c=h)
        return ap[:, ns]

    with tile.TileContext(nc) as tc, \
         tc.tile_pool(name="persist", bufs=1) as pp, \
         tc.tile_pool(name="gxp", bufs=1) as gxp, \
         tc.tile_pool(name="wstream", bufs=6) as wsp, \
         tc.tile_pool(name="stag", bufs=2) as stp, \
         tc.tile_pool(name="gpool", bufs=1) as gp, \
         tc.tile_pool(name="gatep", bufs=1) as gatep, \
         tc.tile_pool(name="tmp", bufs=2) as tp, \
         tc.tile_pool(name="cpool", bufs=2) as cp, \
         tc.tile_pool(name="hpool", bufs=2) as hp, \
         tc.tile_pool(name="psA", bufs=6, space="PSUM") as psA, \
         tc.tile_pool(name="psT", bufs=2, space="PSUM") as psT:

        # ---- persistent tiles ----
        XT = pp.tile([128, kt, w * bl], BF16)      # h^T / layer-input storage
        WHH = pp.tile([128, kt, g4], BF16)         # resident w_hhT
        onesb = pp.tile([1, 512], BF16)
        nc.vector.memset(onesb[:], 1.0)
        idb = pp.tile([64, 64], BF16)
        make_identity(nc, idb[:])
        fc0b_sb = pp.tile([1, h], BF16)
        nc.sync.dma_start(fc0b_sb[:], fc0b_v)
        fc1b_sb = pp.tile([1, h], BF16)
        nc.sync.dma_start(fc1b_sb[:], fc1bT_v)

        # ---- fc0: XT <- fc0_wT.T @ xposT + fc0_b ----
        fc0w = gxp.tile([128, kt, h], BF16, tag="gxE")   # reuses gxE slot space
        nc.sync.dma_start(fc0w[:], fc0wT_v)
        xpt = gxp.tile([128, kt, w * bl], BF16, tag="gxO")  # reuses gxO slot space
        nc.sync.dma_start(xpt[:], xposT_v)
        fc0_chunks = [(c, min(512, w * bl - c)) for c in range(0, w * bl, 512)]
        for m in range(kt):
            for c0, cw in fc0_chunks:
                ps = psA.tile([128, 512], F32, tag="psA", name=f"fc0ps_{m}_{c0}")
                for k in range(kt):
                    nc.tensor.matmul(
                        ps[:, :cw],
                        fc0w[:, k, m * 128:(m + 1) * 128],
                        xpt[:, k, c0:c0 + cw],
                        start=(k == 0), stop=False,
                    )
                nc.tensor.matmul(
                    ps[:, :cw], fc0b_sb[:, m * 128:(m + 1) * 128],
                    onesb[:, 0:cw], start=False, stop=True)
                nc.vector.tensor_copy(XT[:, m, c0:c0 + cw], ps[:, :cw])

        # ---- layers ----
        for l in range(nl):
            nc.sync.dma_start(WHH[:], whhT_v(l))
            bias_sb = pp.tile([1, g4], BF16, tag="bias")
            nc.sync.dma_start(bias_sb[:], biasT_v(l))

            # --- phase-gx ---
            gxE = gxp.tile([64, mt, g4], BF16, tag="gxE")  # even steps (2m)
            gxO = gxp.tile([64, mt, g4], BF16, tag="gxO")  # odd steps (2m+1)
            for j in range(nch):
                js = slice(j * 512, (j + 1) * 512)
                pss = [psA.tile([128, 512], F32, tag="psA", name=f"gxps_{l}_{j}_{m}")
                       for m in range(mt)]
                for k in range(kt):
                    wk = wsp.tile([128, 512], BF16, tag="wch")
                    nc.sync.dma_start(wk[:], wihT_v(l, k, js))
                    for m in range(mt):
                        nc.tensor.matmul(
                            pss[m][:], XT[:, k, m * 128:(m + 1) * 128], wk[:],
                            start=(k == 0), stop=False)
                for m in range(mt):
                    nc.tensor.matmul(
                        pss[m][:], onesb[:, 0:128], bias_sb[:, js], start=False, stop=True)
                for m in range(mt):
                    nc.vector.tensor_copy(gxE[:, m, js], pss[m][0:64, :])
                    stag = stp.tile([128, 512], BF16)
                    nc.vector.tensor_copy(stag[64:128, :], pss[m][64:128, :])
                    nc.sync.dma_start(gxO[:, m, js], stag[64:128, :])

            # --- phase-rec ---
            c_cur = None
            for t in range(w):
                m, half = t // 2, t % 2
                gx_t = (gxE if half == 0 else gxO)[:, m, :]  # (64, g4) bf16
                if t == 0:
                    gsrc = gx_t
                else:
                    g = gp.tile([64, g4], F32)
                    for j in range(nch):
                        js = slice(j * 512, (j + 1) * 512)
                        ps = psA.tile([128, 512], F32, tag="psA", name=f"recps_{l}_{t}_{j}")
                        for k in range(kt):
                            nc.tensor.matmul(
                                ps[0:64, :],
                                XT[:, k, (t - 1) * 64:t * 64],
                                WHH[:, k, js],
                                start=(k == 0), stop=(k == kt - 1))
                        nc.vector.tensor_tensor(
                            g[:, js], ps[0:64, :], gx_t[:, js], OP.add)
                    gsrc = g
                gates = gatep.tile([64, g4], F32)
                TANH = AF.Sigmoid if variant == "allsig" else AF.Tanh
                nc.scalar.activation(gates[:, 0:h], gsrc[:, 0:h], AF.Sigmoid)
                nc.scalar.activation(gates[:, h:2 * h], gsrc[:, h:2 * h], AF.Sigmoid)
                nc.scalar.activation(gates[:, 2 * h:3 * h], gsrc[:, 2 * h:3 * h], TANH)
                nc.scalar.activation(gates[:, 3 * h:4 * h], gsrc[:, 3 * h:4 * h], AF.Sigmoid)
                c_new = cp.tile([64, h], F32)
                if t == 0:
                    nc.vector.tensor_tensor(
                        c_new[:], gates[:, 0:h], gates[:, 2 * h:3 * h], OP.mult)
                else:
                    t1 = tp.tile([64, h], F32, tag="t1")
                    nc.vector.tensor_tensor(
                        t1[:], gates[:, 0:h], gates[:, 2 * h:3 * h], OP.mult)
                    t2 = tp.tile([64, h], F32, tag="t2")
                    nc.vector.tensor_tensor(t2[:], gates[:, h:2 * h], c_cur[:], OP.mult)
                    nc.vector.tensor_tensor(c_new[:], t1[:], t2[:], OP.add)
                c_cur = c_new
                tc_t = tp.tile([64, h], F32, tag="tc")
                nc.scalar.activation(tc_t[:], c_new[:], AF.Sigmoid if variant == "allsig" else AF.Tanh)
                hh = hp.tile([64, h], BF16)
                nc.vector.tensor_tensor(hh[:], gates[:, 3 * h:4 * h], tc_t[:], OP.mult)
                if variant in ("notrans", "noew"):
                    pass  # leave XT stale; timing-structure experiment only
                else:
                    trp = psT.tile([128, kt * 64], BF16)
                    for s in range(kt):
                        nc.tensor.transpose(
                            trp[:, s * 64:(s + 1) * 64], hh[:, s * 128:(s + 1) * 128], idb[:])
                    nc.vector.tensor_copy(
                        XT[:, :, t * 64:(t + 1) * 64],
                        trp[:].rearrange("p (s x) -> p s x", s=kt))

        # ---- fc1 ----
        out_sb = pp.tile([64, h], F32)
        for nchunk in range(2):
            ns = slice(nchunk * fn1, (nchunk + 1) * fn1)
            ps = psA.tile([128, 512], F32, tag="psA", name=f"fc1ps_{nchunk}")
            for ktile in range(w * kt):
                t, s = ktile // kt, ktile % kt
                wk = wsp.tile([128, fn1], BF16, tag="fc1w")
                nc.sync.dma_start(wk[:], fc1wT_v(ktile, ns))
                nc.tensor.matmul(
                    ps[0:64, :fn1], XT[:, s, t * 64:(t + 1) * 64], wk[:],
                    start=(ktile == 0), stop=False)
            nc.tensor.matmul(
                ps[0:64, :fn1], onesb[:, 0:64], fc1b_sb[:, ns], start=False, stop=True)
            nc.vector.tensor_copy(out_sb[:, ns], ps[0:64, :fn1])
        nc.sync.dma_start(out_d[:], out_sb[:])

    nc.compile()
    return nc


def _pack_shared(inputs, h=H, w=W, nl=L):
    """Weights part of the packed buffer (identical across cores), bf16."""
    f32 = np.float32
    bf16 = ml_dtypes.bfloat16
    g4 = 4 * h
    parts = [
        np.ascontiguousarray(np.asarray(inputs["fc0_w"], f32).T).astype(bf16).ravel(),
        np.asarray(inputs["fc0_b"], f32).astype(bf16).ravel(),
        np.ascontiguousarray(
            np.asarray(inputs["w_ih"], f32).transpose(0, 2, 1)).astype(bf16).ravel(),
        np.ascontiguousarray(
            np.asarray(inputs["w_hh"], f32).transpose(0, 2, 1)).astype(bf16).ravel(),
        (np.asarray(inputs["b_ih"], f32) + np.asarray(inputs["b_hh"], f32))
        .astype(bf16).ravel(),
        np.ascontiguousarray(np.asarray(inputs["fc1_w"], f32).T).astype(bf16).ravel(),
        np.asarray(inputs["fc1_b"], f32).astype(bf16).ravel(),
    ]
    return np.concatenate(parts)


def prep_inputs(inputs):
    """Build per-core packed input buffers."""
    bf16 = ml_dtypes.bfloat16
    shared = _pack_shared(inputs)
    x = np.ascontiguousarray(
        np.asarray(inputs["x_position"], np.float32)).reshape(-1, W, H)
    in_maps = []
    for c in range(NCORES):
        xc = x[c * BL:(c + 1) * BL]
        xposT = np.ascontiguousarray(
            xc.transpose(2, 1, 0).reshape(H, W * BL)).astype(bf16).ravel()
        packed = np.empty(TOTAL_ELEMS, bf16)
        packed[:xposT.size] = xposT
        packed[xposT.size:] = shared
        in_maps.append({"packed": packed})
    return in_maps


def run_on_cores(in_maps, trace=False, **kwargs):
    from concourse.bass_utils import run_bass_kernel_spmd
    if "nc" not in _CACHE:
        _CACHE["nc"] = build_program()
    nc = _CACHE["nc"]
    return run_bass_kernel_spmd(
        nc, in_maps, core_ids=list(range(NCORES)), trace=trace, **kwargs)


def kernel(**inputs) -> np.ndarray:
    in_maps = prep_inputs(inputs)
    res = run_on_cores(in_maps)
    outs = [res.results[c]["out"] for c in range(NCORES)]
    full = np.concatenate(outs, axis=0)            # (512, 768)
    return np.ascontiguousarray(full.reshape(-1, 3).astype(np.float32))
